# revision 77
# baseline (speedup 1.0000x reference)
"""Trainium2 Bass kernel for nn_NodeModel (GNN message passing + node MLP), V3.

  agg = scatter_mean(edge_attr, col, N)            # [N, H]
  h   = concat([x, agg]) @ W1 + b1                 # [N, 2H]
  h   = LayerNorm(h) * gamma (+ beta=0)
  h   = PReLU(h)  (single shared a)
  out = h @ W2 + b2                                # [N, H]

V3 strategy (8 cores SPMD, no collectives):
  - Nodes degree-sorted and dealt round-robin so all 8 cores share one
    degree profile D[i] (one program, SPMD).  Padding ~0.03%.
  - Edge attrs shipped fp8 (e4m3) [128, E_pad], slot-major per degree
    class; host scales each edge by 2^k(d)/cnt (k = round(log2 d)) so
    values are O(1) in fp8; the 2^-k is folded into exact-power-of-two
    fp8 diagonal matmul weights.
  - Segment sum on the TENSOR engine: per class, slot pairs are summed
    by fp8 DoubleRow identity matmuls accumulating in PSUM (K=2x128),
    0.25 PE cycles/edge-col; odd leftover slot via a plain fp8 matmul.
    This removes the V2 DVE/GPSIMD add trees entirely and halves the
    dominant edge DMA (fp8 vs f16).
  - MLP feature-major f16: W1 (mean-centered) 4 matmuls; variance via a
    single fp8-DoubleRow ones-matmul over (sq0,sq1); PReLU directly on
    the scalar engine (parametric_relu with alpha operand) reading PSUM;
    W2 2 matmuls; rstd applied after W2 via a partition-broadcast AP on
    the DVE (no broadcast matmul / no gpsimd broadcast needed).
  - rstd = exp(-0.5 ln(var+eps)) as in V2.
"""
import os
import sys

sys.path.insert(0, "/opt/trn_rl_repo")
_HERE = os.path.dirname(os.path.abspath(__file__))
if _HERE not in sys.path:
    sys.path.insert(0, _HERE)

import numpy as np

import concourse.bass as bass
import concourse.tile as tile
from concourse import mybir
from concourse.mybir import AluOpType as alu
from concourse.mybir import ActivationFunctionType as act

F32 = mybir.dt.float32
F16 = mybir.dt.float16
F8 = mybir.dt.float8e4
DR = mybir.MatmulPerfMode.DoubleRow

N_CORES = 8
H = 128
NPC = 12800                    # nodes per core
N_PAD = N_CORES * NPC
MTILE = 512
NMT = NPC // MTILE             # 25
GROUP = 4                      # MLP tiles per stats group
CHUNK = int(os.environ.get("V3_CHUNK", "12288"))   # edge-cols per DMA chunk

# engine assignment knobs (tuned via TimelineSim); env-overridable for sweeps
SQ_ENG = list(os.environ.get("V3_SQ_ENG", "V"))    # sq mult: V=DVE G=pool
AGG_ENG = list(os.environ.get("V3_AGG_ENG", "VA"))   # psum->agg copy rotation
XSLICES = int(os.environ.get("V3_XSLICES", "8"))
BUFS = os.environ.get("V3_BUFS", "2,3,2,1")         # seg,ph,pb,po
CH0 = int(os.environ.get("V3_CH0", "2048"))         # first-chunk size ramp
VSKEW = int(os.environ.get("V3_VSKEW", "3"))        # var lag (tiles)
P2DELAY = int(os.environ.get("V3_P2DELAY", "0"))    # extra phase2 lag

# ---------------------------------------------------------------------------
# walrus workaround (same as V2): single sync-wait per instruction +
# skip the crashy birverifier pass.
import bass_rust


def _split_multi_waits(nc):
    ctr = 0
    for f in nc.m.functions:
        for blk in f.blocks:
            insts = list(blk.instructions)
            new = []
            changed = False
            for inst in insts:
                si = inst.sync_info
                if si is not None and len(si.on_wait) > 1:
                    waits = list(si.on_wait)
                    for w in waits[:-1]:
                        ctr += 1
                        new.append(mybir.InstEventSemaphore(
                            name=f"wsplit_{ctr}", engine=inst.engine,
                            ins=[], outs=[],
                            sync_info=bass_rust.SyncInfo(on_wait=[w],
                                                         on_update=[]),
                        ))
                    si.on_wait = [waits[-1]]
                    changed = True
                new.append(inst)
            if changed:
                blk.instructions = new


def _fuse_single_waits(nc):
    for f in nc.m.functions:
        for blk in f.blocks:
            insts = list(blk.instructions)
            drop = set()
            pending = {}
            for i, inst in enumerate(insts):
                eng = inst.engine
                si = inst.sync_info
                tname = type(inst).__name__
                if (tname == "InstEventSemaphore" and si is not None
                        and len(si.on_wait) == 1 and len(si.on_update) == 0
                        and eng not in pending):
                    pending[eng] = (i, si.on_wait[0])
                    continue
                if eng in pending:
                    if si is not None and len(si.on_wait) > 0:
                        pending.pop(eng)
                    elif tname in ("InstEventSemaphore", "InstDrain",
                                   "InstNoOp", "InstCall", "InstBranch"):
                        pending.pop(eng)
                    else:
                        j, w = pending.pop(eng)
                        if si is None:
                            inst.sync_info = bass_rust.SyncInfo(
                                on_wait=[w], on_update=[])
                        else:
                            si.on_wait = [w]
                        drop.add(j)
            if drop:
                blk.instructions = [x for i, x in enumerate(insts)
                                    if i not in drop]


def _skip_birverifier():
    from concourse import bass_utils as bu
    from pathlib import Path

    if getattr(bu, "_nodemodel_noverify", False):
        return

    def bir_verify_and_optimise(tmpdir, inp="bir.json", outp="file.neff",
                                arch=None, *, dve_root=None):
        cmd = [
            bu.get_walrus_driver(),
            "--pass",
            "runtime_memory_reservation,lower_act,lower_dve,"
            "lower_ap_offset,codegen,neff_packager",
            "-i", inp,
            "--neff-output-filename", outp,
            "--enable-birsim=true",
            "--mem-mode=physical",
            "--policy=0",
            "--enable-ldw-opt=false",
            "--assign-static-dmas-to-sp=false",
            f"--dram-page-size={bu.aot_getenv('NEURON_SCRATCHPAD_PAGE_SIZE', '256')}",
            "--enable-neff-debug-info=true",
            "--jobs", "8",
            *bu.get_walrus_args(
                bu.get_bir_arch(tmpdir, inp) if arch is None else arch,
                tmpdir, dve_root=dve_root),
        ]
        result = bu.run_command(cmd, cwd=tmpdir)
        if result is not None:
            (Path(tmpdir) / "log.txt").write_text(result.stdout)
        return f"{tmpdir}/{outp}"

    bu.bir_verify_and_optimise = bir_verify_and_optimise
    bu._nodemodel_noverify = True


# ---------------------------------------------------------------------------
# weight/constant buffer layout
_O16 = {}
_O8 = {}
_O32 = {}
_NK = 6                         # identity scale tiles for k = 0..5


def _layouts():
    off = 0
    def t16(name, n):
        nonlocal off
        _O16[name] = off
        off += n
    t16("w1a0", 128); t16("w1a1", 128)
    t16("w1b0", 128); t16("w1b1", 128)
    t16("w2g0", 128); t16("w2g1", 128)
    t16("bcw", 512)            # rstd-bcast weights: block r has row 32r ones
    t16("erf", 512)            # 4 f16 ones-tiles [128,97]: col 32r of tile r
    n16 = off
    off = 0
    def t8(name, n):
        nonlocal off
        _O8[name] = off
        off += n
    for k in range(_NK):
        t8(f"i2_{k}", 256)     # DoubleRow identity pair, diag 2^-k
    t8("er2", 4 * 194)         # 4 DR ones-tiles [128, 2, 97], col 32r ones
    n8 = off
    off = 0
    def t32(name, n):
        nonlocal off
        _O32[name] = off
        off += n
    t32("b1c0", 1); t32("b1c1", 1)      # centered bias halves
    t32("gs0", 1); t32("gs1", 1)        # gamma halves (Prelu scale)
    t32("gb0", 1); t32("gb1", 1)        # gamma*b1c halves (Prelu bias)
    t32("alpha", 1)                     # prelu a
    t32("b2c", 1); t32("epsc", 1)
    return n16, n8, off


W16C, W8C, W32C = _layouts()


def _build_wbufs(W1, b1, gamma, beta, prelu_a, W2, b2):
    a = float(np.asarray(prelu_a).reshape(-1)[0])
    W1 = np.asarray(W1, np.float32)
    W2 = np.asarray(W2, np.float32)
    b1 = np.asarray(b1, np.float32)
    b2 = np.asarray(b2, np.float32)
    gamma = np.asarray(gamma, np.float32)
    # center along output features so mean(h) == 0 exactly
    W1c = W1 - W1.mean(axis=1, keepdims=True)
    b1c = b1 - b1.mean()

    w16 = np.zeros((128, W16C), np.float16)
    def s16(name, arr):
        w16[:, _O16[name]:_O16[name] + arr.shape[1]] = arr.astype(np.float16)
    # PReLU(gamma*h*rstd) = rstd * gamma * max(h, a*h) for gamma > 0;
    # fold gamma into W2 (host asserts gamma > 0 in _prepare)
    w2g = W2 * gamma[:, None]
    s16("w1a0", W1c[0:128, 0:128]); s16("w1a1", W1c[0:128, 128:256])
    s16("w1b0", W1c[128:256, 0:128]); s16("w1b1", W1c[128:256, 128:256])
    s16("w2g0", w2g[0:128, :]); s16("w2g1", w2g[128:256, :])
    for r in range(4):
        w16[32 * r, _O16["bcw"] + 128 * r:_O16["bcw"] + 128 * (r + 1)] = 1.0
        w16[:, _O16["erf"] + 128 * r + 32 * r] = 1.0

    w8f = np.zeros((128, W8C), np.float32)
    for k in range(_NK):
        o = _O8[f"i2_{k}"]
        for j in range(2):
            w8f[np.arange(128), o + 128 * j + np.arange(128)] = 2.0 ** (-k)
    for r in range(4):
        o = _O8["er2"] + 194 * r
        w8f[:, o + 32 * r] = 1.0       # j=0 block, col 32r
        w8f[:, o + 97 + 32 * r] = 1.0  # j=1 block, col 32r
    w8 = w8f.astype(mybir.dt.np(F8))

    w32 = np.zeros((128, W32C), np.float32)
    w32[:, _O32["b1c0"]] = b1c[0:128]
    w32[:, _O32["b1c1"]] = b1c[128:256]
    w32[:, _O32["gs0"]] = gamma[0:128]
    w32[:, _O32["gs1"]] = gamma[128:256]
    w32[:, _O32["gb0"]] = gamma[0:128] * b1c[0:128]
    w32[:, _O32["gb1"]] = gamma[128:256] * b1c[128:256]
    w32[:, _O32["alpha"]] = a
    w32[:, _O32["b2c"]] = b2
    w32[:, _O32["epsc"]] = 1e-5
    return w16, w8, w32


# ---------------------------------------------------------------------------
# Structure plan from the shared degree profile D[0..NPC-1] (classes are
# contiguous position runs of equal degree, in profile order).
def _kof(d):
    return min(_NK - 1, max(0, int(round(np.log2(max(d, 1))))))


def _make_plan(D):
    D = np.asarray(D, np.int64)
    assert D.shape == (NPC,)
    classes = []
    p = 0
    while p < NPC:
        d = int(D[p])
        q = p
        while q < NPC and D[q] == d:
            q += 1
        classes.append((d, p, q))
        p = q
    zlo = zhi = 0
    for d, p0, p1 in classes:
        if d == 0:
            zlo, zhi = p0, p1
            break

    # subruns: split each class at MTILE boundaries and so d*n <= CHUNK
    subruns_raw = []           # (d, n, sp, k)
    tot = 0
    for d, p0, p1 in classes:
        if d == 0:
            continue
        k = _kof(d)
        nmax = max(1, CHUNK // d)
        p = p0
        while p < p1:
            lim = min(p1, ((p // MTILE) + 1) * MTILE, p + nmax)
            n = lim - p
            subruns_raw.append((d, n, p, k))
            tot += d * n
            p = lim

    # chunks of consecutive subruns; smaller chunks at both ends (fast
    # pipeline ramp-in and drain).  Chunk starts (and E_pad) are 64B
    # aligned - fp8 gives 1-byte columns, and unaligned DMA source
    # offsets fail on hardware.
    chunks = []                # (src0, ncols, (subrun idx...))
    subruns = []               # (d, n, sp, src, k)
    cur, cur0, cols = [], 0, 0
    src = 0
    done = 0
    def _cap():
        ramp = CH0 * (1 << len(chunks))
        taper = max(CH0 * 2, (tot - done) // 2)
        return max(CH0, min(CHUNK, ramp, taper))
    for i, (d, n, sp, k) in enumerate(subruns_raw):
        c = d * n
        if cur and cols + c > _cap():
            chunks.append((cur0, cols, tuple(cur)))
            done += cols
            cur, cols = [], 0
        if not cur:
            src = -(-src // 64) * 64
            cur0 = src
        subruns.append((d, n, sp, src, k))
        cur.append(i)
        src += c
        cols += c
    if cur:
        chunks.append((cur0, cols, tuple(cur)))
    E_pad = -(-src // 64) * 64

    # per psum-group readiness: last chunk writing positions of group g
    ready_at = np.full(NMT, -1, np.int64)
    for ci, (_, _, idxs) in enumerate(chunks):
        for i in idxs:
            d, n, sp, _, _ = subruns[i]
            g0 = sp // MTILE
            g1 = (sp + n - 1) // MTILE
            for g in range(g0, g1 + 1):
                ready_at[g] = max(ready_at[g], ci)
    # groups fully in the zero-class range stay -1: no psum tile, no copy
    # (their agg range is memset).  Partially-zero groups copy only the
    # written column range [max(g*MTILE, zhi), (g+1)*MTILE).

    # schedule: after each chunk, finished psum groups -> agg copy; MLP
    # tiles (one seg group == one MLP tile) as soon as their agg is ready.
    # "x" entries spread the xT load between the first chunks.
    sched = []
    copied = 0                 # seg groups completed (copied or all-zero)
    t = 0
    for ci in range(len(chunks)):
        sched.append(("c", ci))
        if ci < XSLICES:
            sched.append(("x", ci))
        while copied < NMT and ready_at[copied] <= ci:
            if ready_at[copied] >= 0:
                sched.append(("s", copied))
            copied += 1
        while t < copied:
            sched.append(("t", t))
            t += 1
    for xi in range(len(chunks), XSLICES):
        sched.append(("x", xi))
    while copied < NMT:
        if ready_at[copied] >= 0:
            sched.append(("s", copied))
        copied += 1
    while t < NMT:
        sched.append(("t", t))
        t += 1
    sched.append(("f", 0))

    return dict(
        E_pad=E_pad, zrange=(zlo, zhi),
        classes=tuple(classes),
        subruns=tuple(subruns),
        chunks=tuple(chunks),
        sched=tuple(sched),
        b1z=True, b2z=True,
    )


def _plan_key(plan):
    return (plan["E_pad"], plan["zrange"], plan["subruns"], plan["chunks"],
            plan["sched"], plan["b1z"], plan["b2z"])


# ---------------------------------------------------------------------------
def _build_program(plan, reps=1, unroll=1):
    import contextlib
    _skip_birverifier()
    E_pad = plan["E_pad"]
    zlo, zhi = plan["zrange"]
    subruns, chunks = plan["subruns"], plan["chunks"]
    sched = plan["sched"]
    b1z, b2z = plan["b1z"], plan["b2z"]

    _B = [int(v) for v in BUFS.split(",")]
    nc = bass.Bass("TRN2", target_bir_lowering=False, debug=False,
                   num_devices=N_CORES)
    d_eattr = nc.dram_tensor("eattr", [128, E_pad], F8,
                             kind="ExternalInput").ap()
    d_xT = nc.dram_tensor("xT", [128, NPC], F16, kind="ExternalInput").ap()
    d_w16 = nc.dram_tensor("w16", [128, W16C], F16,
                           kind="ExternalInput").ap()
    d_w8 = nc.dram_tensor("w8", [128, W8C], F8, kind="ExternalInput").ap()
    d_w32 = nc.dram_tensor("w32", [128, W32C], F32,
                           kind="ExternalInput").ap()
    d_outT = nc.dram_tensor("outT", [128, NPC], F16,
                            kind="ExternalOutput").ap()

    with tile.TileContext(nc) as tc:
        with tc.tile_pool(name="const", bufs=1) as constp, \
             tc.tile_pool(name="chp", bufs=3) as chp, \
             tc.tile_pool(name="gp", bufs=12) as gpool, \
             tc.tile_pool(name="sqp", bufs=3) as sqp, \
             tc.tile_pool(name="rowp", bufs=2) as rowp, \
             tc.tile_pool(name="osbp", bufs=2) as osbp, \
             tc.tile_pool(name="ps_seg", bufs=_B[0], space="PSUM") as ps_seg, \
             tc.tile_pool(name="ps_h", bufs=_B[1], space="PSUM") as ps_h, \
             tc.tile_pool(name="ps_pb", bufs=_B[2], space="PSUM") as ps_pb, \
             tc.tile_pool(name="ps_po", bufs=_B[3], space="PSUM") as ps_po:

            w8 = constp.tile([128, W8C], F8)
            nc.sync.dma_start(w8[:], d_w8)        # needed by first chunk
            w16 = constp.tile([128, W16C], F16)
            nc.scalar.dma_start(w16[:], d_w16)
            w32 = constp.tile([128, W32C], F32)
            nc.scalar.dma_start(w32[:], d_w32)
            agg = constp.tile([128, NPC], F16)
            xfull = constp.tile([128, NPC], F16)

            if zhi > zlo:
                nc.gpsimd.memset(agg[:, zlo:zhi], 0.0)

            def W16(name, n=128):
                return w16[:, _O16[name]:_O16[name] + n]

            def W32(name):
                return w32[:, _O32[name]:_O32[name] + 1]

            def I2(k):
                o = _O8[f"i2_{k}"]
                return w8[:, o:o + 256].rearrange("p (two m) -> p two m",
                                                  two=2)

            def I1(k):
                # plain identity with diag 2^-k: j=0 block of the pair tile
                o = _O8[f"i2_{k}"]
                return w8[:, o:o + 128]

            def ER2(r):
                o = _O8["er2"] + 194 * r
                return w8[:, o:o + 194].rearrange("p (two m) -> p two m",
                                                  two=2)

            uid = [0]
            seg_tiles = {}

            def emit_chunk(ci):
                src0, ncols, idxs = chunks[ci]
                uid[0] += 1
                ch = chp.tile([128, CHUNK], F8, name=f"ch{uid[0]}", tag="ch")
                nc.sync.dma_start(ch[:, 0:ncols], d_eattr[:, src0:src0 + ncols])
                for i in idxs:
                    d, n, sp, src, k = subruns[i]
                    g = sp // MTILE
                    st = seg_tiles.get(g)
                    if st is None:
                        st = ps_seg.tile([128, MTILE], F32, name=f"seg{g}",
                                         tag="seg")
                        seg_tiles[g] = [st, set()]
                    stile, written = seg_tiles[g]
                    lo = sp - g * MTILE
                    out = stile[:, lo:lo + n]
                    first = (sp, n) not in written
                    written.add((sp, n))
                    off = src - src0
                    npairs = d // 2
                    for q in range(npairs):
                        rhs = ch[:, off + q * 2 * n: off + (q + 1) * 2 * n]
                        rhs = rhs.rearrange("p (two n) -> p two n", two=2)
                        nc.tensor.matmul(out, I2(k), rhs,
                                         start=(first and q == 0),
                                         stop=(d % 2 == 0 and q == npairs - 1),
                                         perf_mode=DR)
                    if d % 2 == 1:
                        rhs = ch[:, off + (d - 1) * n: off + d * n]
                        nc.tensor.matmul(out, I1(k), rhs,
                                         start=(first and npairs == 0),
                                         stop=True)

            def emit_seg_copy(g):
                stile, _ = seg_tiles.pop(g)
                lo = max(g * MTILE, zhi)       # skip memset zero-class cols
                n = (g + 1) * MTILE - lo
                sl = slice(lo, lo + n)
                psl = stile[:, lo - g * MTILE:lo - g * MTILE + n]
                eng = AGG_ENG[g % len(AGG_ENG)]
                if eng == "V":
                    nc.vector.tensor_copy(agg[:, sl], psl)
                else:
                    nc.scalar.activation(agg[:, sl], psl, act.Copy)

            # MLP pipeline state (reset per rep emission)
            mlp = {}

            def emit_tile(m):
                sl = slice(m * MTILE, (m + 1) * MTILE)
                xt = xfull[:, sl]
                aggm = agg[:, sl]
                ph0 = ps_h.tile([128, MTILE], F32, tag="ph", name=f"ph0_{m}")
                ph1 = ps_h.tile([128, MTILE], F32, tag="ph", name=f"ph1_{m}")
                nc.tensor.matmul(ph0[:], W16("w1a0"), xt, start=True,
                                 stop=False)
                nc.tensor.matmul(ph0[:], W16("w1b0"), aggm, start=False,
                                 stop=True)
                nc.tensor.matmul(ph1[:], W16("w1a1"), xt, start=True,
                                 stop=False)
                nc.tensor.matmul(ph1[:], W16("w1b1"), aggm, start=False,
                                 stop=True)
                # single cheap psum reader per half: frees the ph banks fast
                h01 = gpool.tile([128, 2 * MTILE], F16, tag="h",
                                 name=f"h_{m}")
                nc.scalar.activation(h01[:, 0:MTILE], ph0[:], act.Identity,
                                     bias=W32("b1c0"))
                nc.scalar.activation(h01[:, MTILE:], ph1[:], act.Identity,
                                     bias=W32("b1c1"))
                # PReLU: g = max(h, a*h)  (gamma folded into W2); the Pool
                # engine only compiles add/mult TensorTensor ops here, so
                # the max lives on the DVE (scalar_tensor_tensor)
                g01 = gpool.tile([128, 2 * MTILE], F16, tag="g",
                                 name=f"g_{m}")
                nc.vector.scalar_tensor_tensor(
                    g01[:], h01[:], W32("alpha"), h01[:],
                    alu.mult, alu.max)
                # sq = h*h (f16, feeds the f16 variance matmuls)
                sq = sqp.tile([128, 2 * MTILE], F16, tag="sq",
                              name=f"sq_{m}")
                if SQ_ENG[m % len(SQ_ENG)] == "G":
                    nc.gpsimd.tensor_tensor(sq[:], h01[:], h01[:], alu.mult)
                else:
                    nc.vector.tensor_tensor(sq[:], h01[:], h01[:], alu.mult)
                mlp[m] = (g01, sq)

            def emit_var(m):
                gi = m // GROUP
                r = m % GROUP
                G = min(GROUP, NMT - gi * GROUP)
                if r == 0:
                    mlp[("pb", gi)] = ps_pb.tile([128, MTILE], F32, tag="pb",
                                                 name=f"pb_{gi}")
                pb = mlp[("pb", gi)]
                sq = mlp[m][1]
                er = w16[:, _O16["erf"] + 128 * r:_O16["erf"] + 128 * r + 97]
                nc.tensor.matmul(pb[0:97, :], er, sq[:, 0:MTILE],
                                 start=(r == 0), stop=False)
                nc.tensor.matmul(pb[0:97, :], er, sq[:, MTILE:],
                                 start=False, stop=(r == G - 1))

            def emit_phase2(gi):
                m0 = GROUP * gi
                G = min(GROUP, NMT - m0)
                pb = mlp.pop(("pb", gi))   # reused below as bcast scratch
                np97 = 32 * (G - 1) + 1
                yrow = rowp.tile([128, MTILE], F32, tag="yr", name=f"yr_{gi}")
                nc.scalar.activation(yrow[0:np97, :], pb[0:np97, :], act.Ln,
                                     scale=1.0 / 256.0,
                                     bias=w32[0:np97, _O32["epsc"]:
                                              _O32["epsc"] + 1])
                rrow = rowp.tile([128, MTILE], F16, tag="rr", name=f"rr_{gi}")
                nc.scalar.activation(rrow[0:np97, :], yrow[0:np97, :],
                                     act.Exp, scale=-0.5)
                osb = osbp.tile([128, GROUP * MTILE], F16, tag="osb",
                                name=f"osb_{gi}")
                for r in range(G):
                    m = m0 + r
                    g01, _sq = mlp.pop(m)
                    po = ps_po.tile([128, MTILE], F32, tag="po",
                                    name=f"po_{m}")
                    nc.tensor.matmul(po[:], W16("w2g0"), g01[:, 0:MTILE],
                                     start=True, stop=False)
                    nc.tensor.matmul(po[:], W16("w2g1"), g01[:, MTILE:],
                                     start=False, stop=True)
                    # rstd broadcast: ones-row matmul into a rotating psum
                    # bank, then a cheap copy to SBUF f16.  base partition 96
                    # is not addressable, so r=3 uses a K=33 slice from 64.
                    bc = _O16["bcw"] + 128 * r
                    if r < 3:
                        lhsT = w16[32 * r:32 * r + 1, bc:bc + 128]
                        rhs = rrow[32 * r:32 * r + 1, :]
                    else:
                        lhsT = w16[64:97, bc:bc + 128]
                        rhs = rrow[64:97, :]
                    nc.tensor.matmul(pb[:], lhsT, rhs, start=True, stop=True)
                    rb = gpool.tile([128, MTILE], F16, tag="rb",
                                    name=f"rb_{m}")
                    nc.scalar.activation(rb[:], pb[:], act.Copy)
                    dst = osb[:, r * MTILE:(r + 1) * MTILE]
                    nc.vector.tensor_tensor(dst, po[:], rb[:], alu.mult)
                    if not b2z:
                        nc.vector.tensor_scalar(dst, dst, W32("b2c"),
                                                None, alu.add)
                nc.scalar.dma_start(
                    d_outT[:, m0 * MTILE:m0 * MTILE + G * MTILE],
                    osb[:, 0:G * MTILE])

            def emit_rep():
                mlp.clear()
                mlp["next_var"] = 0
                mlp["p2q"] = []

                def advance(limit, tpos):
                    # emit var for tiles < limit; queue phase2 when a
                    # group's vars are all in; drain queue with delay
                    while mlp["next_var"] < limit:
                        mv = mlp["next_var"]
                        emit_var(mv)
                        gi = mv // GROUP
                        G = min(GROUP, NMT - gi * GROUP)
                        if mv % GROUP == G - 1:
                            mlp["p2q"].append((gi, mv + P2DELAY))
                        mlp["next_var"] = mv + 1
                    while mlp["p2q"] and mlp["p2q"][0][1] <= tpos:
                        emit_phase2(mlp["p2q"].pop(0)[0])

                for kind_it, idx in sched:
                    if kind_it == "c":
                        emit_chunk(idx)
                    elif kind_it == "x":
                        xw = NPC // XSLICES
                        sl = slice(idx * xw, (idx + 1) * xw)
                        nc.scalar.dma_start(xfull[:, sl], d_xT[:, sl])
                    elif kind_it == "s":
                        emit_seg_copy(idx)
                    elif kind_it == "t":
                        emit_tile(idx)
                        advance(idx - VSKEW + 1, idx)
                    else:
                        advance(NMT, NMT + GROUP + P2DELAY)   # flush

            with nc.allow_low_precision("f16/fp8 pipeline"):
                if unroll > 1:
                    for _ in range(unroll):
                        emit_rep()
                else:
                    rep_ctx = (tc.For_i(0, reps, 1) if reps > 1
                               else contextlib.nullcontext())
                    rep_ctx.__enter__()
                    emit_rep()
                    rep_ctx.__exit__(None, None, None)

    _split_multi_waits(nc)
    _fuse_single_waits(nc)
    return nc


# ---------------------------------------------------------------------------
class _Runner:
    """Persistent executor: jit once, keep inputs on device."""

    def __init__(self, nc):
        import jax
        from jax.experimental.shard_map import shard_map
        from jax.sharding import Mesh, PartitionSpec, NamedSharding
        from concourse import bass2jax
        from concourse import mybir as _mb

        bass2jax.install_neuronx_cc_hook()
        self.nc = nc
        in_names, out_names, out_avals = [], [], []
        partition_name = (nc.partition_id_tensor.name
                          if nc.partition_id_tensor else None)
        for alloc in nc.m.functions[0].allocations:
            if not isinstance(alloc, _mb.MemoryLocationSet):
                continue
            name = alloc.memorylocations[0].name
            if alloc.kind == "ExternalInput":
                if name != partition_name:
                    in_names.append(name)
            elif alloc.kind == "ExternalOutput":
                out_names.append(name)
                out_avals.append(jax.core.ShapedArray(
                    tuple(alloc.tensor_shape), _mb.dt.np(alloc.dtype)))
        self.in_names, self.out_names, self.out_avals = \
            in_names, out_names, out_avals
        n_params, n_outs = len(in_names), len(out_avals)
        all_in = list(in_names) + list(out_names)
        if partition_name is not None:
            all_in.append(partition_name)

        def _body(*args):
            operands = list(args)
            if partition_name is not None:
                operands.append(bass2jax.partition_id_tensor())
            return tuple(bass2jax._bass_exec_p.bind(
                *operands,
                out_avals=tuple(out_avals),
                in_names=tuple(all_in),
                out_names=tuple(out_names),
                lowering_input_output_aliases=(),
                sim_require_finite=True,
                sim_require_nnan=True,
                nc=nc,
            ))

        devices = jax.devices()[:N_CORES]
        mesh = Mesh(np.asarray(devices), ("core",))
        self.mesh = mesh
        self.sharding = NamedSharding(mesh, PartitionSpec("core"))
        in_specs = (PartitionSpec("core"),) * (n_params + n_outs)
        out_specs = (PartitionSpec("core"),) * n_outs
        donate = tuple(range(n_params, n_params + n_outs))
        self.fn = jax.jit(
            shard_map(_body, mesh=mesh, in_specs=in_specs,
                      out_specs=out_specs, check_rep=False),
            donate_argnums=donate, keep_unused=True)
        self._zero = jax.jit(
            lambda: tuple(
                jax.numpy.zeros((N_CORES * a.shape[0], *a.shape[1:]), a.dtype)
                for a in out_avals),
            out_shardings=tuple(self.sharding for _ in out_avals))
        self._dev_inputs = None
        self._dev_key = None

    def put_inputs(self, in_maps):
        import jax
        key = tuple(id(m[n]) for m in in_maps for n in self.in_names)
        if self._dev_key == key and self._dev_inputs is not None:
            return
        concat = [np.concatenate([np.asarray(m[n]) for m in in_maps], axis=0)
                  for n in self.in_names]
        self._dev_inputs = [jax.device_put(a, self.sharding) for a in concat]
        for a in self._dev_inputs:
            a.block_until_ready()
        self._dev_key = key

    def execute(self):
        zeros = self._zero()
        outs = self.fn(*self._dev_inputs, *zeros)
        return outs

    def run(self, in_maps):
        self.put_inputs(in_maps)
        outs = self.execute()
        res = []
        for c in range(N_CORES):
            res.append({
                name: np.asarray(outs[i]).reshape(
                    N_CORES, *self.out_avals[i].shape)[c]
                for i, name in enumerate(self.out_names)})
        return res

    def time_once(self):
        import time as _t
        zeros = self._zero()
        for z in zeros:
            z.block_until_ready()
        t0 = _t.perf_counter()
        outs = self.fn(*self._dev_inputs, *zeros)
        for o in outs:
            o.block_until_ready()
        return _t.perf_counter() - t0


_CACHE = {}


def _prepare(x, edge_index, edge_attr, W1, b1, gamma, beta, prelu_a, W2, b2):
    N, E = x.shape[0], edge_attr.shape[0]
    assert np.all(np.asarray(beta) == 0.0), "kernel specialized for beta=0"
    assert np.all(np.asarray(gamma) > 0.0), "kernel specialized for gamma>0"
    x = np.asarray(x, np.float32)
    edge_attr = np.asarray(edge_attr, np.float32)
    col = np.asarray(edge_index)[1].astype(np.int64)

    cnt = np.bincount(col, minlength=N_PAD).astype(np.int64)

    # order positions: class rank ascending degree, zero-degree first
    dmax = int(cnt.max())
    rank = np.arange(dmax + 1)
    order = np.argsort(rank[cnt], kind="stable")
    r = np.arange(N_PAD)
    new_of_old = np.empty(N_PAD, np.int64)
    new_of_old[order] = (r % N_CORES) * NPC + r // N_CORES
    # shared profile: max degree within each 8-block = last of the block
    D = cnt[order].reshape(NPC, N_CORES).max(axis=1)

    plan = _make_plan(D)
    plan["b1z"] = bool(np.all(np.asarray(b1) == 0.0))
    plan["b2z"] = bool(np.all(np.asarray(b2) == 0.0))
    E_pad = plan["E_pad"]

    # per-edge (core, pos, slot)
    new = new_of_old[col]
    core_e = new // NPC
    pos_e = new % NPC
    order_e = np.argsort(new, kind="stable")
    sn = new[order_e]
    change = np.r_[True, sn[1:] != sn[:-1]]
    startidx = np.maximum.accumulate(np.where(change, np.arange(E), 0))
    slot = np.empty(E, np.int64)
    slot[order_e] = np.arange(E) - startidx

    # per-position subrun lookup: src base and (sp, n) of its subrun
    src_of_pos = np.zeros(NPC, np.int64)
    n_of_pos = np.zeros(NPC, np.int64)
    sp_of_pos = np.zeros(NPC, np.int64)
    k_of_pos = np.zeros(NPC, np.int64)
    for d, n, sp, src, k in plan["subruns"]:
        src_of_pos[sp:sp + n] = src
        n_of_pos[sp:sp + n] = n
        sp_of_pos[sp:sp + n] = sp
        k_of_pos[sp:sp + n] = k
    colx = (src_of_pos[pos_e] + slot * n_of_pos[pos_e]
            + (pos_e - sp_of_pos[pos_e]))

    # host scale: 2^k(class) / cnt(node); exact 2^-k folded into weights
    scale = (2.0 ** k_of_pos[pos_e].astype(np.float64)
             / np.maximum(cnt[col], 1)).astype(np.float32)
    val = (edge_attr * scale[:, None]).astype(mybir.dt.np(F8))
    buf = np.zeros((N_CORES, E_pad, H), mybir.dt.np(F8))
    buf[core_e, colx] = val
    eattrT = np.ascontiguousarray(buf.transpose(0, 2, 1))

    xp = np.zeros((N_PAD, H), np.float32)
    xp[new_of_old[:N]] = x
    xT = np.ascontiguousarray(
        xp.reshape(N_CORES, NPC, H).transpose(0, 2, 1)).astype(np.float16)

    w16, w8, w32 = _build_wbufs(W1, b1, gamma, beta, prelu_a, W2, b2)

    in_maps = [
        {"eattr": eattrT[c], "xT": xT[c], "w16": w16, "w8": w8, "w32": w32}
        for c in range(N_CORES)
    ]
    return plan, in_maps, new_of_old


def get_runner(plan, reps=1):
    ck = (_plan_key(plan), reps)
    runner = _CACHE.get(ck)
    if runner is None:
        nc = _build_program(plan, reps=reps)
        runner = _Runner(nc)
        _CACHE[ck] = runner
    return runner


def kernel(x, edge_index, edge_attr, W1, b1, gamma, beta, prelu_a, W2, b2,
           **_unused):
    N = x.shape[0]
    plan, in_maps, new_of_old = _prepare(x, edge_index, edge_attr, W1, b1,
                                         gamma, beta, prelu_a, W2, b2)
    runner = get_runner(plan)
    res = runner.run(in_maps)
    outT = np.stack([r["outT"] for r in res])           # [8,128,NPC] f16
    out = outT.transpose(0, 2, 1).reshape(N_PAD, H).astype(np.float32)
    out = out[new_of_old[:N]]
    return np.ascontiguousarray(out)


if __name__ == "__main__":
    rng = np.random.default_rng(0)
    N, E = N_PAD, 60000
    x = rng.standard_normal((N, H), dtype=np.float32)
    ei = rng.integers(0, N, size=(2, E)).astype(np.int64)
    ea = rng.standard_normal((E, H), dtype=np.float32)
    W1 = rng.standard_normal((2 * H, 2 * H), dtype=np.float32) / 16
    b1 = np.zeros(2 * H, np.float32)
    g = np.ones(2 * H, np.float32)
    be = np.zeros(2 * H, np.float32)
    a = np.full(1, 0.25, np.float32)
    W2 = rng.standard_normal((2 * H, H), dtype=np.float32) / 16
    b2 = np.zeros(H, np.float32)
    out = kernel(x, ei, ea, W1, b1, g, be, a, W2, b2)
    print("out", out.shape, out.dtype, np.abs(out).mean())
